# revision 1
# baseline (speedup 1.0000x reference)
"""Trainium2 Bass kernel for nn_DescrptSeT (DeepMD three-body descriptor).

Self-contained: hardcodes shapes from the problem spec.
  nlist (1,256,96) i32, extended_coord (1,1536) f32, extended_atype (1,512) i32,
  mean/stddev (2,96,4) f32, W1 (3,1,24), b1 (3,24), W2 (3,24,48), b2 (3,48),
  W3 (3,48,96), b3 (3,96) -> out (1,256,96) f32.

Strategy (8 cores, data-parallel over the 256 local atoms, 32 per core):
  stage 1: gather neighbor coords (indirect DMA), build rr = diff*sw/l^2 in a
           (128 x 24) wide layout, apply (rr-mean)/std, pair matmuls
           env_p = rr_i^T rr_j on PE.
  stage 2: flatten env into a 4-lane MLP batch (feat-major) + a batch-major
           one-hot layout for the contraction. MLP 1->24->48->96 with tanh and
           resnet duplication residuals:
             L1 as K=4 blockdiag matmul, tanh on ACT (bias per-partition),
             L2 as 4 packed K=24 matmuls into a spread feature layout
                (features at 32-aligned rows; "ones row" made by tanh(30)=1),
             residual h2 = T2 + [T1;T1] via wide DVE adds (T1 duplicated by DMA),
             L3 flipped: lhsT = h2-chunk (65 x 128) stationary, rhs = [W3|I] so
                one matmul yields batch-major [P3^T | h2^T],
             contraction on PE: lhsT = one-hot env columns (128 x 8), PSUM-
                accumulated per (atom,pair) segment window.
  stage 3: selection-matrix matmul folds windows*scales -> (32 x 144), final
           DVE add folds the [rh2;rh2] duplication, DMA out.
"""

import os
import sys

sys.path.insert(0, "/opt/trn_rl_repo")

import numpy as np

import concourse.bass as bass
import concourse.tile as tile
from concourse import bacc, mybir
from concourse.bass_utils import run_bass_kernel_spmd

F32 = mybir.dt.float32
BF16 = mybir.dt.bfloat16
I32 = mybir.dt.int32

USE_BF16 = os.environ.get("KERNEL_BF16", "0") == "1"
DT = BF16 if USE_BF16 else F32
NPDT = np.dtype("bfloat16") if USE_BF16 else np.float32

# problem constants
NCORES = 8
NLOC, NALL, NNEI, NG = 256, 512, 96, 96
SEL = [32, 64]
PAIRS = [(0, 0), (0, 1), (1, 1)]
PAIR_SC = [1.0 / (SEL[ti] * SEL[tj]) for ti, tj in PAIRS]
RCUT, RCUT_SMTH = 6.0, 0.5

A_CORE = NLOC // NCORES            # 32 atoms per core
PER_ATOM = 32 * 32 + 32 * 64 + 64 * 64   # 7168 env elems per atom
B_CORE = A_CORE * PER_ATOM         # 229376
LANES = 4
LANE_COLS = B_CORE // LANES        # 57344 (8 atoms per lane)
NSUPER = LANE_COLS // 512          # 112
CC_LANE = LANE_COLS // 128         # 448 chunks per lane
POFF = [0, 1024, 3072]             # pair segment offsets within an atom
PLEN = [1024, 2048, 4096]
# pair index per supertile position within an atom's 14 supertiles
PAIR_OF_POS = [0] * 2 + [1] * 4 + [2] * 8


def _pair_of_cc(ccm):  # ccm = cc % 56 (chunk within atom)
    if ccm < 8:
        return 0
    if ccm < 24:
        return 1
    return 2


def _sid_of_cc(cc):
    return 3 * (cc // 56) + _pair_of_cc(cc % 56)


# window tables (windows pair up consecutive sids; all lanes share the
# schedule). For window w: first lane-chunk W0[w], length NCHW[w].
SEG_NCH = [PLEN[p] // 128 for p in range(3)]         # 8, 16, 32
SID_START = []                                        # lane-chunk of sid start
for _sid in range(24):
    SID_START.append(56 * (_sid // 3) + POFF[_sid % 3] // 128)
W0 = [SID_START[2 * w] for w in range(12)]
NCHW = [SEG_NCH[(2 * w) % 3] + SEG_NCH[(2 * w + 1) % 3] for w in range(12)]
WBASE = [8 * sum(NCHW[:w]) for w in range(12)]        # env_bm col base of window


def _win_of_cc(cc):
    return _sid_of_cc(cc) // 2


# ---------------------------------------------------------------- host aux

def _build_static_aux(W1, b1, W2, b2, W3, b3, mean, stddev):
    """Input-weight-derived aux tensors (replicated to all cores)."""
    aux = {}
    # W1rep (3, 128, 128): quarter-q rows 32q+g hold W1 blockdiag row g
    w1blk = np.zeros((3, 4, 128), np.float32)
    for p in range(3):
        for g in range(4):
            w1blk[p, g, 32 * g:32 * g + 24] = W1[p, 0, :]
    w1rep = np.zeros((3, 128, 128), np.float32)
    for q in range(4):
        w1rep[:, 32 * q:32 * q + 4, :] = w1blk
    aux["w1rep"] = w1rep
    # b1t (3, 128, 1)
    b1t = np.zeros((3, 128, 1), np.float32)
    for g in range(4):
        b1t[:, 32 * g:32 * g + 24, 0] = b1
    aux["b1t"] = b1t
    # W2 aux: even groups -> out rows 0:64 (feats {0:24,32:56}),
    #         odd groups  -> out rows 64:128 (col0=0, feats {8:32,40:64})
    w2A = np.zeros((3, 128, 64), np.float32)
    w2B = np.zeros((3, 128, 64), np.float32)
    for g in range(4):
        r = slice(32 * g, 32 * g + 24)
        w2A[:, r, 0:24] = W2[:, :, 0:24]
        w2A[:, r, 32:56] = W2[:, :, 24:48]
        w2B[:, r, 8:32] = W2[:, :, 0:24]
        w2B[:, r, 40:64] = W2[:, :, 24:48]
    aux["w2A"] = w2A.astype(NPDT)
    aux["w2B"] = w2B.astype(NPDT)
    # b2 spread bias (3, 128, 1): rows {0:24,32:56} = b2 (even half),
    # row 64 = 30 (tanh -> exact 1.0 ones row), rows {72:96,104:128} = b2 (odd)
    b2sp = np.zeros((3, 128, 1), np.float32)
    b2sp[:, 0:24, 0] = b2[:, 0:24]
    b2sp[:, 32:56, 0] = b2[:, 24:48]
    b2sp[:, 64, 0] = 30.0
    b2sp[:, 72:96, 0] = b2[:, 0:24]
    b2sp[:, 104:128, 0] = b2[:, 24:48]
    aux["b2sp"] = b2sp
    # W3cat A (65 x 144) for even half, B (64 x 144) for odd half
    w3A = np.zeros((3, 65, 144), np.float32)
    w3B = np.zeros((3, 64, 144), np.float32)
    for p in range(3):
        w3A[p, 0:24, 0:96] = W3[p, 0:24, :]
        w3A[p, 32:56, 0:96] = W3[p, 24:48, :]
        w3A[p, 64, 0:96] = b3[p]
        w3A[p, 0:24, 96:120] = np.eye(24)
        w3A[p, 32:56, 120:144] = np.eye(24)
        # odd half: tile rows 64+r, r=0 is the shared ones row
        w3B[p, 0, 0:96] = b3[p]
        w3B[p, 8:32, 0:96] = W3[p, 0:24, :]
        w3B[p, 40:64, 0:96] = W3[p, 24:48, :]
        w3B[p, 8:32, 96:120] = np.eye(24)
        w3B[p, 40:64, 120:144] = np.eye(24)
    aux["w3A"] = w3A.astype(NPDT)
    aux["w3B"] = w3B.astype(NPDT)
    # SEL (96, 32): row 4*sid+l, col 8*l + sid//3, value = pair scale
    sel = np.zeros((96, 32), np.float32)
    for sid in range(24):
        for l in range(4):
            sel[4 * sid + l, 8 * l + sid // 3] = PAIR_SC[sid % 3]
    aux["sel"] = sel
    # mean/std tables (8, 72): row t*4+q, col 24*c+f -> slot q*24+f, coord 1+c
    mA = np.zeros((8, 72), np.float32)
    mB = np.zeros((8, 72), np.float32)
    for t in range(2):
        for q in range(4):
            for c in range(3):
                sl = slice(24 * c, 24 * c + 24)
                mA[t * 4 + q, sl] = 1.0 / stddev[t, q * 24:q * 24 + 24, 1 + c]
                mB[t * 4 + q, sl] = (mean[t, q * 24:q * 24 + 24, 1 + c]
                                     / stddev[t, q * 24:q * 24 + 24, 1 + c])
    aux["mstabA"] = mA
    aux["mstabB"] = mB
    aux["qvec"] = (np.arange(128, dtype=np.float32) % 4).reshape(128, 1)
    # selection matrices for env_bm construction: S_k1[r, j] = 1 iff
    # r == k1c*j + k1, replicated at the 32-aligned row bases
    def srep(R, k1c, nbases):
        nch = R // k1c
        out = np.zeros((k1c, 128, nch), np.float32)
        for k1 in range(k1c):
            s = np.zeros((R, nch), np.float32)
            for j in range(nch):
                s[k1c * j + k1, j] = 1.0
            for b in range(nbases):
                out[k1, (128 // nbases) * b:(128 // nbases) * b + R, :] = s
        return out
    aux["s0rep"] = srep(32, 4, 4)
    aux["s1rep"] = srep(32, 2, 4)
    aux["s2rep"] = srep(64, 2, 2)
    return aux


# ---------------------------------------------------------------- program

def build_program():
    nc = bacc.Bacc("TRN2", target_bir_lowering=False, debug=False,
                   enable_asserts=True, num_devices=NCORES)

    # DRAM IO (per-core values supplied via in_maps)
    coordT = nc.dram_tensor("coordT", [NALL, 3], F32, kind="ExternalInput").ap()
    atypeF = nc.dram_tensor("atypeF", [NALL, 1], F32, kind="ExternalInput").ap()
    nlist24 = nc.dram_tensor("nlist24", [128, 24], I32, kind="ExternalInput").ap()
    ownid = nc.dram_tensor("ownid", [128, 1], I32, kind="ExternalInput").ap()
    qvecD = nc.dram_tensor("qvec", [128, 1], F32, kind="ExternalInput").ap()
    mstabAD = nc.dram_tensor("mstabA", [8, 72], F32, kind="ExternalInput").ap()
    mstabBD = nc.dram_tensor("mstabB", [8, 72], F32, kind="ExternalInput").ap()
    w1repD = nc.dram_tensor("w1rep", [3, 128, 128], F32, kind="ExternalInput").ap()
    b1tD = nc.dram_tensor("b1t", [3, 128, 1], F32, kind="ExternalInput").ap()
    w2AD = nc.dram_tensor("w2A", [3, 128, 64], DT, kind="ExternalInput").ap()
    w2BD = nc.dram_tensor("w2B", [3, 128, 64], DT, kind="ExternalInput").ap()
    b2spD = nc.dram_tensor("b2sp", [3, 128, 1], F32, kind="ExternalInput").ap()
    w3AD = nc.dram_tensor("w3A", [3, 65, 144], DT, kind="ExternalInput").ap()
    w3BD = nc.dram_tensor("w3B", [3, 64, 144], DT, kind="ExternalInput").ap()
    selD = nc.dram_tensor("sel", [96, 32], F32, kind="ExternalInput").ap()
    s0repD = nc.dram_tensor("s0rep", [4, 128, 8], F32, kind="ExternalInput").ap()
    s1repD = nc.dram_tensor("s1rep", [2, 128, 16], F32, kind="ExternalInput").ap()
    s2repD = nc.dram_tensor("s2rep", [2, 128, 32], F32, kind="ExternalInput").ap()
    outD = nc.dram_tensor("out", [A_CORE, NG], F32, kind="ExternalOutput").ap()

    TANH = mybir.ActivationFunctionType.Tanh
    SQRT = mybir.ActivationFunctionType.Sqrt
    COPYF = mybir.ActivationFunctionType.Copy
    MUL = mybir.AluOpType.mult
    SUB = mybir.AluOpType.subtract
    ADD = mybir.AluOpType.add

    with tile.TileContext(nc) as tc, \
         tc.tile_pool(name="wpool", bufs=1) as wpool, \
         tc.tile_pool(name="s1", bufs=1) as s1, \
         tc.tile_pool(name="sbT1", bufs=2) as sbT1, \
         tc.tile_pool(name="sbT1d", bufs=2) as sbT1d, \
         tc.tile_pool(name="sbT2", bufs=4) as sbT2, \
         tc.tile_pool(name="sbTT", bufs=4) as sbTT:

        # ---- persistent weights in SBUF
        def wtile(ap_dram, shape, dtype, tag):
            t = wpool.tile(shape, dtype, tag=tag)
            nc.sync.dma_start(t[:], ap_dram)
            return t

        w1sb = [wtile(w1repD[p], [128, 128], F32, f"w1_{p}") for p in range(3)]
        b1sb = [wtile(b1tD[p], [128, 1], F32, f"b1_{p}") for p in range(3)]
        w2Asb = [wtile(w2AD[p], [128, 64], DT, f"w2a_{p}") for p in range(3)]
        w2Bsb = [wtile(w2BD[p], [128, 64], DT, f"w2b_{p}") for p in range(3)]
        b2sb = [wtile(b2spD[p], [128, 1], F32, f"b2_{p}") for p in range(3)]
        w3Asb = [wtile(w3AD[p], [65, 144], DT, f"w3a_{p}") for p in range(3)]
        # W3B lives at partitions 64:128 so the odd-half flip matmul's lhsT
        # (T2sp rows 64:128) and rhs share a partition base
        w3Bsb = []
        for p in range(3):
            t = wpool.tile([128, 144], DT, tag=f"w3b_{p}", name=f"w3b_{p}")
            nc.sync.dma_start(t[64:128, :], w3BD[p])
            w3Bsb.append(t)
        selsb = wtile(selD, [96, 32], F32, "sel")
        s0rep = [wtile(s0repD[k], [128, 8], F32, f"s0_{k}") for k in range(4)]
        s1rep = [wtile(s1repD[k], [128, 16], F32, f"s1_{k}") for k in range(2)]
        s2rep = [wtile(s2repD[k], [128, 32], F32, f"s2_{k}") for k in range(2)]
        st_all = wpool.tile([96, 144], F32, tag="stall")

        # ---- stage 1: env construction
        with tc.tile_pool(name="ps1", bufs=2, space="PSUM") as ps1, \
             tc.tile_pool(name="psbm", bufs=2, space="PSUM") as psbm:
            nl = s1.tile([128, 24], I32, tag="nl")
            nc.sync.dma_start(nl[:], nlist24)
            oid = s1.tile([128, 1], I32, tag="oid")
            nc.sync.dma_start(oid[:], ownid)
            qv = s1.tile([128, 1], F32, tag="qv")
            nc.sync.dma_start(qv[:], qvecD)

            nbr = s1.tile([128, 72], F32, tag="nbr")
            for f in range(24):
                nc.gpsimd.indirect_dma_start(
                    out=nbr[:, 3 * f:3 * f + 3], out_offset=None, in_=coordT,
                    in_offset=bass.IndirectOffsetOnAxis(ap=nl[:, f:f + 1], axis=0))
            own = s1.tile([128, 3], F32, tag="own")
            nc.gpsimd.indirect_dma_start(
                out=own[:], out_offset=None, in_=coordT,
                in_offset=bass.IndirectOffsetOnAxis(ap=oid[:, 0:1], axis=0))
            tvec = s1.tile([128, 1], F32, tag="tvec")
            nc.gpsimd.indirect_dma_start(
                out=tvec[:], out_offset=None, in_=atypeF,
                in_offset=bass.IndirectOffsetOnAxis(ap=oid[:, 0:1], axis=0))
            # trow = 4*type + q  (as int32 for the gather)
            trowf = s1.tile([128, 1], F32, tag="trowf")
            nc.vector.tensor_scalar(out=trowf[:], in0=tvec[:], scalar1=4.0,
                                    scalar2=None, op0=MUL)
            nc.vector.tensor_add(trowf[:], trowf[:], qv[:])
            trow = s1.tile([128, 1], I32, tag="trow")
            nc.vector.tensor_copy(trow[:], trowf[:])
            Aexp = s1.tile([128, 72], F32, tag="Aexp")
            nc.gpsimd.indirect_dma_start(
                out=Aexp[:], out_offset=None, in_=mstabAD,
                in_offset=bass.IndirectOffsetOnAxis(ap=trow[:, 0:1], axis=0))
            Bexp = s1.tile([128, 72], F32, tag="Bexp")
            nc.gpsimd.indirect_dma_start(
                out=Bexp[:], out_offset=None, in_=mstabBD,
                in_offset=bass.IndirectOffsetOnAxis(ap=trow[:, 0:1], axis=0))

            d = [s1.tile([128, 24], F32, tag=f"d{c}", name=f"d{c}")
                 for c in range(3)]
            nbr3 = nbr[:].rearrange("p (f c) -> p f c", c=3)
            for c in range(3):
                nc.vector.tensor_tensor(
                    out=d[c][:], in0=nbr3[:, :, c],
                    in1=own[:, c:c + 1].to_broadcast([128, 24]), op=SUB)
            l2 = s1.tile([128, 24], F32, tag="l2")
            tmp = s1.tile([128, 24], F32, tag="tmp")
            nc.vector.tensor_tensor(out=l2[:], in0=d[0][:], in1=d[0][:], op=MUL)
            nc.vector.tensor_tensor(out=tmp[:], in0=d[1][:], in1=d[1][:], op=MUL)
            nc.vector.tensor_add(l2[:], l2[:], tmp[:])
            nc.vector.tensor_tensor(out=tmp[:], in0=d[2][:], in1=d[2][:], op=MUL)
            nc.vector.tensor_add(l2[:], l2[:], tmp[:])
            ll = s1.tile([128, 24], F32, tag="ll")
            nc.scalar.activation(ll[:], l2[:], SQRT)
            rinv2 = s1.tile([128, 24], F32, tag="rinv2")
            nc.vector.reciprocal(rinv2[:], l2[:])
            # smooth weight: uu=(l-rmin)/(rmax-rmin) clamped to [0,1],
            # vv = uu^3(-6uu^2+15uu-10)+1
            uu = s1.tile([128, 24], F32, tag="uu")
            sc = 1.0 / (RCUT - RCUT_SMTH)
            nc.scalar.activation(uu[:], ll[:], COPYF, bias=-RCUT_SMTH * sc, scale=sc)
            nc.vector.tensor_scalar(out=uu[:], in0=uu[:], scalar1=0.0, scalar2=1.0,
                                    op0=mybir.AluOpType.max, op1=mybir.AluOpType.min)
            poly = s1.tile([128, 24], F32, tag="poly")
            nc.vector.tensor_scalar(out=poly[:], in0=uu[:], scalar1=-6.0,
                                    scalar2=15.0, op0=MUL, op1=ADD)
            nc.vector.tensor_tensor(out=poly[:], in0=poly[:], in1=uu[:], op=MUL)
            nc.vector.tensor_scalar(out=poly[:], in0=poly[:], scalar1=10.0,
                                    scalar2=None, op0=SUB)
            u2 = s1.tile([128, 24], F32, tag="u2")
            nc.vector.tensor_tensor(out=u2[:], in0=uu[:], in1=uu[:], op=MUL)
            nc.vector.tensor_tensor(out=u2[:], in0=u2[:], in1=uu[:], op=MUL)
            nc.vector.tensor_tensor(out=poly[:], in0=poly[:], in1=u2[:], op=MUL)
            nc.vector.tensor_scalar(out=poly[:], in0=poly[:], scalar1=1.0,
                                    scalar2=None, op0=ADD)
            # s2 = sw / l^2 ; rr_c = (d_c * s2) * A_c - B_c
            s2 = s1.tile([128, 24], F32, tag="s2")
            nc.vector.tensor_tensor(out=s2[:], in0=poly[:], in1=rinv2[:], op=MUL)
            rrf = s1.tile([3, 3072], F32, tag="rrf")
            for c in range(3):
                rr = s1.tile([128, 24], F32, tag="rr")
                nc.vector.tensor_tensor(out=rr[:], in0=d[c][:], in1=s2[:], op=MUL)
                nc.vector.tensor_tensor(out=rr[:], in0=rr[:],
                                        in1=Aexp[:, 24 * c:24 * c + 24], op=MUL)
                nc.vector.tensor_tensor(out=rr[:], in0=rr[:],
                                        in1=Bexp[:, 24 * c:24 * c + 24], op=SUB)
                nc.sync.dma_start(rrf[c:c + 1, :], rr[:])

            # pair matmuls -> envA (128 x 768) [4 atoms x (32 x 96) = env00|env01],
            #                 envB (128 x 1024) [2 atoms x (64 x 64) = env11]
            envA = s1.tile([128, 768], F32, tag="envA")
            envB = s1.tile([128, 1024], F32, tag="envB")
            for blk in range(8):          # 4-atom blocks
                pa = ps1.tile([128, 96], F32, tag="pa")
                for j in range(4):
                    a = 4 * blk + j
                    nc.tensor.matmul(
                        out=pa[32 * j:32 * j + 32, :],
                        lhsT=rrf[:, 96 * a:96 * a + 32],
                        rhs=rrf[:, 96 * a:96 * a + 96], start=True, stop=True,
                        tile_position=(0, 32 * j))
                nc.vector.tensor_copy(envA[:, 96 * blk:96 * blk + 96], pa[:])
            for blk in range(16):         # 2-atom blocks
                pb = ps1.tile([128, 64], F32, tag="pb")
                for j in range(2):
                    a = 2 * blk + j
                    nc.tensor.matmul(
                        out=pb[64 * j:64 * j + 64, :],
                        lhsT=rrf[:, 96 * a + 32:96 * a + 96],
                        rhs=rrf[:, 96 * a + 32:96 * a + 96], start=True,
                        stop=True, tile_position=(0, 64 * j))
                nc.vector.tensor_copy(envB[:, 64 * blk:64 * blk + 64], pb[:])

            # ---- flatten into the MLP batch order (plain row-major within
            # each (atom, pair) segment).
            env_mlp = s1.tile([128, 14336], F32, tag="env_mlp")
            for q in range(4):
                for l in range(4):
                    row = 32 * q + l
                    for aa in range(2):
                        a = 8 * l + 2 * q + aa
                        base = 7168 * aa
                        srcs = (
                            (0, envA[32 * (a % 4):32 * (a % 4) + 32,
                                     96 * (a // 4):96 * (a // 4) + 32]),
                            (1024, envA[32 * (a % 4):32 * (a % 4) + 32,
                                        96 * (a // 4) + 32:
                                        96 * (a // 4) + 96]),
                            (3072, envB[64 * (a % 2):64 * (a % 2) + 64,
                                        64 * (a // 2):64 * (a // 2) + 64]),
                        )
                        for off, src in srcs:
                            seglen = src.shape[0] * src.shape[1]
                            nc.sync.dma_start(
                                env_mlp[row:row + 1, base + off:
                                        base + off + seglen], src)

            # ---- env_bm per lane (128 x 3584), contiguous window sections:
            # window w occupies cols [WBASE[w], WBASE[w]+8*NCHW[w]); section r
            # holds the chunks whose acc row is r, at their window position.
            # Built on PE: chunk-column block = tile^T @ S_k1 (selection),
            # with the normal env tile as lhsT (psum rows C*k1 via col tiling).
            env_bm = []
            for l in range(4):
                eb = s1.tile([128, 3584], F32, tag=f"env_bm{l}",
                             name=f"env_bm{l}")
                nc.vector.memset(eb[:], 0.0)
                env_bm.append(eb)
            for l in range(4):
                for ap_ in range(8):        # atom within lane
                    a = 8 * l + ap_
                    for P in range(3):
                        sid = 3 * ap_ + P
                        w = sid // 2
                        nch = SEG_NCH[P]
                        r = 4 * (sid % 2) + l
                        col0 = (WBASE[w] + r * NCHW[w]
                                + SID_START[sid] - W0[w])
                        if P == 0:
                            tile_ = envA[32 * (a % 4):32 * (a % 4) + 32,
                                         96 * (a // 4):96 * (a // 4) + 32]
                            k1c, C, rb = 4, 32, 32 * (a % 4)
                            sreps = s0rep
                        elif P == 1:
                            tile_ = envA[32 * (a % 4):32 * (a % 4) + 32,
                                         96 * (a // 4) + 32:96 * (a // 4) + 96]
                            k1c, C, rb = 2, 64, 32 * (a % 4)
                            sreps = s1rep
                        else:
                            tile_ = envB[64 * (a % 2):64 * (a % 2) + 64,
                                         64 * (a // 2):64 * (a // 2) + 64]
                            k1c, C, rb = 2, 64, 64 * (a % 2)
                            sreps = s2rep
                        R = tile_.shape[0]
                        pbm = psbm.tile([128, nch], F32, tag="pbm", name="pbm")
                        for kk in range(k1c):
                            nc.tensor.matmul(
                                out=pbm[C * kk:C * kk + C, :],
                                lhsT=tile_,
                                rhs=sreps[kk][rb:rb + R, :],
                                start=True, stop=True,
                                tile_position=(rb, C * kk))
                        nc.vector.tensor_copy(env_bm[l][:, col0:col0 + nch],
                                              pbm[:])

        # ---- stage 2: MLP + contraction over 112 supertiles
        with tc.tile_pool(name="psL1", bufs=1, space="PSUM") as psL1, \
             tc.tile_pool(name="psL2", bufs=2, space="PSUM") as psL2, \
             tc.tile_pool(name="psP3", bufs=2, space="PSUM") as psP3, \
             tc.tile_pool(name="psacc", bufs=1, space="PSUM") as psacc:

            acc_tile = None
            win_open = -1
            for s in range(NSUPER):
                q, si = s // 28, s % 28
                P = PAIR_OF_POS[s % 14]
                p1 = psL1.tile([128, 512], F32, tag="p1")
                nc.tensor.matmul(out=p1[:],
                                 lhsT=w1sb[P][32 * q:32 * q + 4, :],
                                 rhs=env_mlp[32 * q:32 * q + 4,
                                             512 * si:512 * si + 512],
                                 start=True, stop=True,
                                 tile_position=(32 * q, 0))
                t1 = sbT1.tile([128, 512], DT, tag="t1")
                nc.scalar.activation(t1[:], p1[:], TANH, bias=b1sb[P][:, 0:1])
                # T1dup (128 x 1024): cols 512t, rows: {0:32,32:64}<-T1[64t:+32],
                # {64:104}<-T1[64t+24:+40], {104:128}<-T1[64t+32:+24]
                t1d = sbT1d.tile([128, 1024], DT, tag="t1d")
                for t in range(2):
                    for (dr0, nr, sr0) in ((0, 32, 0), (32, 32, 0),
                                           (64, 40, 24), (104, 24, 32)):
                        nc.sync.dma_start(
                            t1d[dr0:dr0 + nr, 512 * t:512 * t + 512],
                            t1[64 * t + sr0:64 * t + sr0 + nr, :])
                t2s = []
                for t in range(2):
                    p2 = psL2.tile([128, 512], F32, tag="p2")
                    ge, go = 2 * t, 2 * t + 1
                    nc.tensor.matmul(out=p2[0:64, :],
                                     lhsT=w2Asb[P][32 * ge:32 * ge + 24, :],
                                     rhs=t1[32 * ge:32 * ge + 24, :],
                                     start=True, stop=True,
                                     tile_position=(32 * ge, 0))
                    nc.tensor.matmul(out=p2[64:128, :],
                                     lhsT=w2Bsb[P][32 * go:32 * go + 24, :],
                                     rhs=t1[32 * go:32 * go + 24, :],
                                     start=True, stop=True,
                                     tile_position=(32 * go, 64))
                    t2 = sbT2.tile([128, 512], DT, tag="t2")
                    nc.scalar.activation(t2[:], p2[:], TANH, bias=b2sb[P][:, 0:1])
                    nc.vector.tensor_add(t2[0:64, :], t2[0:64, :],
                                         t1d[0:64, 512 * t:512 * t + 512])
                    nc.vector.tensor_add(t2[64:128, :], t2[64:128, :],
                                         t1d[64:128, 512 * t:512 * t + 512])
                    t2s.append(t2)
                for l in range(4):
                    t, par = l // 2, l % 2
                    p3 = psP3.tile([128, 1024], F32, tag="p3")
                    for qq in range(4):
                        if par == 0:
                            nc.tensor.matmul(
                                out=p3[:, 256 * qq:256 * qq + 144],
                                lhsT=t2s[t][0:65, 128 * qq:128 * qq + 128],
                                rhs=w3Asb[P][:], start=True, stop=True)
                        else:
                            nc.tensor.matmul(
                                out=p3[:, 256 * qq:256 * qq + 144],
                                lhsT=t2s[t][64:128, 128 * qq:128 * qq + 128],
                                rhs=w3Bsb[P][64:128, :], start=True, stop=True)
                    tt = sbTT.tile([128, 576], F32, tag="tt")
                    p3r = p3[:].rearrange("p (k x) -> p k x", k=4)
                    ttr = tt[:].rearrange("p (k x) -> p k x", k=4)
                    nc.scalar.activation(ttr[:, :, 0:96], p3r[:, :, 0:96], TANH)
                    nc.vector.tensor_copy(ttr[:, :, 96:144], p3r[:, :, 96:144])
                    for qq in range(4):
                        cc = 4 * s + qq
                        sid = _sid_of_cc(cc)
                        w = sid // 2
                        if w != win_open:
                            # flush previous window via SBUF staging
                            if acc_tile is not None:
                                stg = sbTT.tile([8, 144], F32, tag="stg",
                                                name="stg")
                                nc.vector.tensor_copy(stg[:], acc_tile[:])
                                nc.sync.dma_start(
                                    st_all[8 * win_open:8 * win_open + 8, :],
                                    stg[:])
                            acc_tile = psacc.tile([8, 144], F32, tag="acc")
                            win_open = w
                            first = True
                        else:
                            first = False
                        last = (sid % 2 == 1) and (cc == 56 * (sid // 3)
                                                   + (POFF[sid % 3] + PLEN[sid % 3]) // 128 - 1) \
                            and (l == 3)
                        # lhsT: 8 one-hot cols = window sections at this
                        # chunk's position (stride NCHW[w])
                        pos = cc - W0[w]
                        lhs = env_bm[l][:, WBASE[w]:WBASE[w] + 8 * NCHW[w]] \
                            .rearrange("p (r j) -> p j r", r=8)[:, pos, :]
                        nc.tensor.matmul(
                            out=acc_tile[:], lhsT=lhs,
                            rhs=tt[:, 144 * qq:144 * qq + 144],
                            start=first, stop=last, skip_group_check=True)
            # flush last window
            stg = sbTT.tile([8, 144], F32, tag="stg", name="stg")
            nc.vector.tensor_copy(stg[:], acc_tile[:])
            nc.sync.dma_start(st_all[8 * win_open:8 * win_open + 8, :], stg[:])

            # ---- stage 3: combine
            res_ps = psL1.tile([32, 144], F32, tag="p1")
            nc.tensor.matmul(out=res_ps[:], lhsT=selsb[:], rhs=st_all[:],
                             start=True, stop=True)
            res_cp = wpool.tile([32, 144], F32, tag="rescp")
            nc.vector.tensor_copy(res_cp[:], res_ps[:])
            res_sb = wpool.tile([32, 96], F32, tag="res")
            nc.vector.tensor_tensor(
                out=res_sb[:].rearrange("p (r f) -> p r f", r=2),
                in0=res_cp[:, 0:96].rearrange("p (r f) -> p r f", r=2),
                in1=res_cp[:, 96:144].rearrange("p (r f) -> p r f", r=1)
                    .to_broadcast([32, 2, 48]),
                op=ADD)
            nc.sync.dma_start(outD, res_sb[:])

    nc.compile()
    return nc


_CACHE = {}


def _get_program():
    if "nc" not in _CACHE:
        _CACHE["nc"] = build_program()
    return _CACHE["nc"]


def make_in_maps(nlist, extended_coord, extended_atype, mean, stddev,
                 W1, b1, W2, b2, W3, b3):
    nlist = np.asarray(nlist)
    aux = _build_static_aux(np.asarray(W1, np.float32), np.asarray(b1, np.float32),
                            np.asarray(W2, np.float32), np.asarray(b2, np.float32),
                            np.asarray(W3, np.float32), np.asarray(b3, np.float32),
                            np.asarray(mean, np.float32),
                            np.asarray(stddev, np.float32))
    coordT = np.asarray(extended_coord, np.float32).reshape(NALL, 3)
    atypeF = np.asarray(extended_atype).astype(np.float32).reshape(NALL, 1)

    in_maps = []
    for c in range(NCORES):
        m = {
            "coordT": coordT,
            "atypeF": atypeF,
            "nlist24": nlist[0, 32 * c:32 * c + 32, :].astype(np.int32)
                       .reshape(128, 24),
            "ownid": (32 * c + np.arange(128) // 4).astype(np.int32)
                     .reshape(128, 1),
            "qvec": aux["qvec"],
            "mstabA": aux["mstabA"], "mstabB": aux["mstabB"],
            "sel": aux["sel"],
            "s0rep": aux["s0rep"], "s1rep": aux["s1rep"],
            "s2rep": aux["s2rep"],
            "w1rep": aux["w1rep"], "b1t": aux["b1t"],
            "w2A": aux["w2A"], "w2B": aux["w2B"], "b2sp": aux["b2sp"],
            "w3A": aux["w3A"], "w3B": aux["w3B"],
        }
        in_maps.append(m)
    return in_maps


def kernel_run(trace=False, **inputs):
    in_maps = make_in_maps(**inputs)
    nc = _get_program()
    res = run_bass_kernel_spmd(nc, in_maps, core_ids=list(range(NCORES)),
                               trace=trace)
    out = np.concatenate([res.results[c]["out"] for c in range(NCORES)], axis=0)
    return out.reshape(1, NLOC, NG).astype(np.float32), res


def kernel(nlist, extended_coord, extended_atype, mean, stddev,
           W1, b1, W2, b2, W3, b3):
    out, _ = kernel_run(
        nlist=nlist, extended_coord=extended_coord,
        extended_atype=extended_atype, mean=mean, stddev=stddev,
        W1=W1, b1=b1, W2=W2, b2=b2, W3=W3, b3=b3)
    return out



# revision 26
# speedup vs baseline: 2.6061x; 2.6061x over previous
"""Trainium2 Bass kernel for nn_DescrptSeT (DeepMD three-body descriptor).

Self-contained: hardcodes shapes from the problem spec.
  nlist (1,256,96) i32, extended_coord (1,1536) f32, extended_atype (1,512) i32,
  mean/stddev (2,96,4) f32, W1 (3,1,24), b1 (3,24), W2 (3,24,48), b2 (3,48),
  W3 (3,48,96), b3 (3,96) -> out (1,256,96) f32.

Strategy (8 cores, data-parallel over the 256 local atoms, 32 per core):
  stage 1: gather neighbor coords (indirect DMA), build rr = diff*sw/l^2 in a
           (128 x 24) wide layout, apply (rr-mean)/std, pair matmuls
           env_p = rr_i^T rr_j on PE.
  stage 2: flatten env into a 4-lane MLP batch (feat-major) + a batch-major
           one-hot layout for the contraction. MLP 1->24->48->96 with tanh and
           resnet duplication residuals:
             L1 as K=4 blockdiag matmul, tanh on ACT (bias per-partition),
             L2 as 4 packed K=24 matmuls into a spread feature layout
                (features at 32-aligned rows; "ones row" made by tanh(30)=1),
             residual h2 = T2 + [T1;T1] via wide DVE adds (T1 duplicated by DMA),
             L3 flipped: lhsT = h2-chunk (65 x 128) stationary, rhs = [W3|I] so
                one matmul yields batch-major [P3^T | h2^T],
             contraction on PE: lhsT = one-hot env columns (128 x 8), PSUM-
                accumulated per (atom,pair) segment window.
  stage 3: selection-matrix matmul folds windows*scales -> (32 x 144), final
           DVE add folds the [rh2;rh2] duplication, DMA out.
"""

import os
import sys

sys.path.insert(0, "/opt/trn_rl_repo")

import numpy as np

import concourse.bass as bass
import concourse.tile as tile
from concourse import bacc, mybir
from concourse.bass_utils import run_bass_kernel_spmd

F32 = mybir.dt.float32
BF16 = mybir.dt.bfloat16
I32 = mybir.dt.int32

USE_BF16 = os.environ.get("KERNEL_BF16", "1") == "1"
DT = BF16 if USE_BF16 else F32
NPDT = np.dtype("bfloat16") if USE_BF16 else np.float32

# problem constants
NCORES = 8
NLOC, NALL, NNEI, NG = 256, 512, 96, 96
SEL = [32, 64]
PAIRS = [(0, 0), (0, 1), (1, 1)]
PAIR_SC = [1.0 / (SEL[ti] * SEL[tj]) for ti, tj in PAIRS]
RCUT, RCUT_SMTH = 6.0, 0.5

A_CORE = NLOC // NCORES            # 32 atoms per core
PER_ATOM = 32 * 32 + 32 * 64 + 64 * 64   # 7168 env elems per atom
B_CORE = A_CORE * PER_ATOM         # 229376
LANES = 4
LANE_COLS = B_CORE // LANES        # 57344 (8 atoms per lane)
NSUPER = LANE_COLS // 512          # 112
CC_LANE = LANE_COLS // 128         # 448 chunks per lane
POFF = [0, 1024, 3072]             # pair segment offsets within an atom
PLEN = [1024, 2048, 4096]
# pair index per supertile position within an atom's 14 supertiles
PAIR_OF_POS = [0] * 2 + [1] * 4 + [2] * 8


def _pair_of_cc(ccm):  # ccm = cc % 56 (chunk within atom)
    if ccm < 8:
        return 0
    if ccm < 24:
        return 1
    return 2


def _sid_of_cc(cc):
    return 3 * (cc // 56) + _pair_of_cc(cc % 56)


# window tables (windows pair up consecutive sids; all lanes share the
# schedule). For window w: first lane-chunk W0[w], length NCHW[w].
SEG_NCH = [PLEN[p] // 128 for p in range(3)]         # 8, 16, 32
SID_START = []                                        # lane-chunk of sid start
for _sid in range(24):
    SID_START.append(56 * (_sid // 3) + POFF[_sid % 3] // 128)
W0 = [SID_START[2 * w] for w in range(12)]
NCHW = [SEG_NCH[(2 * w) % 3] + SEG_NCH[(2 * w + 1) % 3] for w in range(12)]
WBASE = [8 * sum(NCHW[:w]) for w in range(12)]        # env_bm col base of window


def _win_of_cc(cc):
    return _sid_of_cc(cc) // 2


# ---------------------------------------------------------------- host aux

def _build_static_aux(W1, b1, W2, b2, W3, b3, mean, stddev):
    """Input-weight-derived aux tensors (replicated to all cores)."""
    aux = {}
    # W1rep (3, 128, 128): quarter-q rows 32q+g hold W1 blockdiag row g
    w1blk = np.zeros((3, 4, 128), np.float32)
    for p in range(3):
        for g in range(4):
            w1blk[p, g, 32 * g:32 * g + 24] = W1[p, 0, :]
    w1rep = np.zeros((3, 128, 128), np.float32)
    for q in range(4):
        w1rep[:, 32 * q:32 * q + 4, :] = w1blk
    aux["w1rep"] = w1rep.astype(NPDT)
    # b1t (3, 128, 1)
    b1t = np.zeros((3, 128, 1), np.float32)
    for g in range(4):
        b1t[:, 32 * g:32 * g + 24, 0] = b1
    aux["b1t"] = b1t
    # Fused W2 (3, 128, 128): one K=64 matmul per t1 half computes both the
    # even elem (out rows 0:64, feats {0:24,32:56}) and the odd elem
    # (out rows 64:128, feats {72:96,104:128}; row 64 stays 0 for the
    # b2sp tanh(30)=1 ones row).
    w2F = np.zeros((3, 128, 128), np.float32)
    for g in range(4):
        r = slice(32 * g, 32 * g + 24)
        if g % 2 == 0:
            w2F[:, r, 0:24] = W2[:, :, 0:24]
            w2F[:, r, 32:56] = W2[:, :, 24:48]
        else:
            w2F[:, r, 72:96] = W2[:, :, 0:24]
            w2F[:, r, 104:128] = W2[:, :, 24:48]
    aux["w2F"] = w2F.astype(NPDT)
    # b2 spread bias (3, 128, 1): rows {0:24,32:56} = b2 (even half),
    # row 64 = 30 (tanh -> exact 1.0 ones row), rows {72:96,104:128} = b2 (odd)
    b2sp = np.zeros((3, 128, 1), np.float32)
    b2sp[:, 0:24, 0] = b2[:, 0:24]
    b2sp[:, 32:56, 0] = b2[:, 24:48]
    b2sp[:, 64, 0] = 30.0
    b2sp[:, 72:96, 0] = b2[:, 0:24]
    b2sp[:, 104:128, 0] = b2[:, 24:48]
    aux["b2sp"] = b2sp
    # W3cat A (65 x 144) for even half, B (64 x 144) for odd half
    w3A = np.zeros((3, 65, 144), np.float32)
    w3B = np.zeros((3, 64, 144), np.float32)
    for p in range(3):
        w3A[p, 0:24, 0:96] = W3[p, 0:24, :]
        w3A[p, 32:56, 0:96] = W3[p, 24:48, :]
        w3A[p, 64, 0:96] = b3[p]
        w3A[p, 0:24, 96:120] = np.eye(24)
        w3A[p, 32:56, 120:144] = np.eye(24)
        # odd half: tile rows 64+r, r=0 is the shared ones row
        w3B[p, 0, 0:96] = b3[p]
        w3B[p, 8:32, 0:96] = W3[p, 0:24, :]
        w3B[p, 40:64, 0:96] = W3[p, 24:48, :]
        w3B[p, 8:32, 96:120] = np.eye(24)
        w3B[p, 40:64, 120:144] = np.eye(24)
    aux["w3A"] = w3A.astype(NPDT)
    aux["w3B"] = w3B.astype(NPDT)
    # SEL (96, 32): row 4*sid+l, col 8*l + sid//3, value = pair scale
    sel = np.zeros((96, 32), np.float32)
    for sid in range(24):
        for l in range(4):
            sel[4 * sid + l, 8 * l + sid // 3] = PAIR_SC[sid % 3]
    aux["sel"] = sel
    # mean/std tables (8, 72): row t*4+q, col 24*c+f -> slot q*24+f, coord 1+c
    mA = np.zeros((8, 72), np.float32)
    mB = np.zeros((8, 72), np.float32)
    for t in range(2):
        for q in range(4):
            for c in range(3):
                sl = slice(24 * c, 24 * c + 24)
                mA[t * 4 + q, sl] = 1.0 / stddev[t, q * 24:q * 24 + 24, 1 + c]
                mB[t * 4 + q, sl] = (mean[t, q * 24:q * 24 + 24, 1 + c]
                                     / stddev[t, q * 24:q * 24 + 24, 1 + c])
    aux["mstabA"] = mA
    aux["mstabB"] = mB
    aux["qvec"] = (np.arange(128, dtype=np.float32) % 4).reshape(128, 1)
    # selection matrices for env_bm construction: S_k1[r, j] = 1 iff
    # r == k1c*j + k1, replicated at the 32-aligned row bases
    def srep(R, k1c, nbases):
        nch = R // k1c
        out = np.zeros((k1c, 128, nch), np.float32)
        for k1 in range(k1c):
            s = np.zeros((R, nch), np.float32)
            for j in range(nch):
                s[k1c * j + k1, j] = 1.0
            for b in range(nbases):
                out[k1, (128 // nbases) * b:(128 // nbases) * b + R, :] = s
        return out
    aux["s0rep"] = srep(32, 4, 4).astype(NPDT)
    aux["s1rep"] = srep(32, 2, 4).astype(NPDT)
    aux["s2rep"] = srep(64, 2, 2).astype(NPDT)
    return aux


# ---------------------------------------------------------------- program

def build_program():
    nc = bacc.Bacc("TRN2", target_bir_lowering=False, debug=False,
                   enable_asserts=True, num_devices=NCORES)

    # DRAM IO (per-core values supplied via in_maps)
    coordT = nc.dram_tensor("coordT", [NALL, 3], F32, kind="ExternalInput").ap()
    atypeF = nc.dram_tensor("atypeF", [NALL, 1], F32, kind="ExternalInput").ap()
    nlist24 = nc.dram_tensor("nlist24", [128, 24], I32, kind="ExternalInput").ap()
    ownid = nc.dram_tensor("ownid", [128, 1], I32, kind="ExternalInput").ap()
    qvecD = nc.dram_tensor("qvec", [128, 1], F32, kind="ExternalInput").ap()
    mstabAD = nc.dram_tensor("mstabA", [8, 72], F32, kind="ExternalInput").ap()
    mstabBD = nc.dram_tensor("mstabB", [8, 72], F32, kind="ExternalInput").ap()
    w1repD = nc.dram_tensor("w1rep", [3, 128, 128], DT, kind="ExternalInput").ap()
    b1tD = nc.dram_tensor("b1t", [3, 128, 1], F32, kind="ExternalInput").ap()
    w2FD = nc.dram_tensor("w2F", [3, 128, 128], DT, kind="ExternalInput").ap()
    b2spD = nc.dram_tensor("b2sp", [3, 128, 1], F32, kind="ExternalInput").ap()
    w3AD = nc.dram_tensor("w3A", [3, 65, 144], DT, kind="ExternalInput").ap()
    w3BD = nc.dram_tensor("w3B", [3, 64, 144], DT, kind="ExternalInput").ap()
    selD = nc.dram_tensor("sel", [96, 32], F32, kind="ExternalInput").ap()
    s0repD = nc.dram_tensor("s0rep", [4, 128, 8], DT, kind="ExternalInput").ap()
    s1repD = nc.dram_tensor("s1rep", [2, 128, 16], DT, kind="ExternalInput").ap()
    s2repD = nc.dram_tensor("s2rep", [2, 128, 32], DT, kind="ExternalInput").ap()
    outD = nc.dram_tensor("out", [A_CORE, NG], F32, kind="ExternalOutput").ap()

    TANH = mybir.ActivationFunctionType.Tanh
    SQRT = mybir.ActivationFunctionType.Sqrt
    COPYF = mybir.ActivationFunctionType.Copy
    MUL = mybir.AluOpType.mult
    SUB = mybir.AluOpType.subtract
    ADD = mybir.AluOpType.add

    with tile.TileContext(nc) as tc, \
         tc.tile_pool(name="wpool", bufs=1) as wpool, \
         tc.tile_pool(name="s1", bufs=1) as s1, \
         tc.tile_pool(name="sbT1", bufs=2) as sbT1, \
         tc.tile_pool(name="sbT1d", bufs=2) as sbT1d, \
         tc.tile_pool(name="sbT2", bufs=4) as sbT2, \
         tc.tile_pool(name="sbTT", bufs=4) as sbTT:

        # ---- persistent weights in SBUF
        def wtile(ap_dram, shape, dtype, tag):
            t = wpool.tile(shape, dtype, tag=tag)
            nc.sync.dma_start(t[:], ap_dram)
            return t

        w1sb = [wtile(w1repD[p], [128, 128], DT, f"w1_{p}") for p in range(3)]
        b1sb = [wtile(b1tD[p], [128, 1], F32, f"b1_{p}") for p in range(3)]
        w2Fsb = [wtile(w2FD[p], [128, 128], DT, f"w2f_{p}") for p in range(3)]
        b2sb = [wtile(b2spD[p], [128, 1], F32, f"b2_{p}") for p in range(3)]
        w3Asb = [wtile(w3AD[p], [65, 144], DT, f"w3a_{p}") for p in range(3)]
        # W3B lives at partitions 64:128 so the odd-half flip matmul's lhsT
        # (T2sp rows 64:128) and rhs share a partition base
        w3Bsb = []
        for p in range(3):
            t = wpool.tile([128, 144], DT, tag=f"w3b_{p}", name=f"w3b_{p}")
            nc.sync.dma_start(t[64:128, :], w3BD[p])
            w3Bsb.append(t)
        selsb = wtile(selD, [96, 32], F32, "sel")
        s0rep = [wtile(s0repD[k], [128, 8], DT, f"s0_{k}") for k in range(4)]
        s1rep = [wtile(s1repD[k], [128, 16], DT, f"s1_{k}") for k in range(2)]
        s2rep = [wtile(s2repD[k], [128, 32], DT, f"s2_{k}") for k in range(2)]
        st_all = wpool.tile([96, 144], F32, tag="stall")

        # ---- stage 1: env construction
        with tc.tile_pool(name="ps1", bufs=2, space="PSUM") as ps1, \
             tc.tile_pool(name="psbm", bufs=2, space="PSUM") as psbm:
            nl = s1.tile([128, 24], I32, tag="nl")
            nc.sync.dma_start(nl[:], nlist24)
            oid = s1.tile([128, 1], I32, tag="oid")
            nc.sync.dma_start(oid[:], ownid)
            qv = s1.tile([128, 1], F32, tag="qv")
            nc.sync.dma_start(qv[:], qvecD)

            nbr = s1.tile([128, 72], F32, tag="nbr")
            for f in range(24):
                nc.gpsimd.indirect_dma_start(
                    out=nbr[:, 3 * f:3 * f + 3], out_offset=None, in_=coordT,
                    in_offset=bass.IndirectOffsetOnAxis(ap=nl[:, f:f + 1], axis=0))
            own = s1.tile([128, 3], F32, tag="own")
            nc.gpsimd.indirect_dma_start(
                out=own[:], out_offset=None, in_=coordT,
                in_offset=bass.IndirectOffsetOnAxis(ap=oid[:, 0:1], axis=0))
            tvec = s1.tile([128, 1], F32, tag="tvec")
            nc.gpsimd.indirect_dma_start(
                out=tvec[:], out_offset=None, in_=atypeF,
                in_offset=bass.IndirectOffsetOnAxis(ap=oid[:, 0:1], axis=0))
            # trow = 4*type + q  (as int32 for the gather)
            trowf = s1.tile([128, 1], F32, tag="trowf")
            nc.vector.tensor_scalar(out=trowf[:], in0=tvec[:], scalar1=4.0,
                                    scalar2=None, op0=MUL)
            nc.vector.tensor_add(trowf[:], trowf[:], qv[:])
            trow = s1.tile([128, 1], I32, tag="trow")
            nc.vector.tensor_copy(trow[:], trowf[:])
            Aexp = s1.tile([128, 72], F32, tag="Aexp")
            nc.gpsimd.indirect_dma_start(
                out=Aexp[:], out_offset=None, in_=mstabAD,
                in_offset=bass.IndirectOffsetOnAxis(ap=trow[:, 0:1], axis=0))
            Bexp = s1.tile([128, 72], F32, tag="Bexp")
            nc.gpsimd.indirect_dma_start(
                out=Bexp[:], out_offset=None, in_=mstabBD,
                in_offset=bass.IndirectOffsetOnAxis(ap=trow[:, 0:1], axis=0))

            d = [s1.tile([128, 24], F32, tag=f"d{c}", name=f"d{c}")
                 for c in range(3)]
            nbr3 = nbr[:].rearrange("p (f c) -> p f c", c=3)
            for c in range(3):
                nc.vector.tensor_tensor(
                    out=d[c][:], in0=nbr3[:, :, c],
                    in1=own[:, c:c + 1].to_broadcast([128, 24]), op=SUB)
            l2 = s1.tile([128, 24], F32, tag="l2")
            tmp = s1.tile([128, 24], F32, tag="tmp")
            nc.vector.tensor_tensor(out=l2[:], in0=d[0][:], in1=d[0][:], op=MUL)
            nc.vector.tensor_tensor(out=tmp[:], in0=d[1][:], in1=d[1][:], op=MUL)
            nc.vector.tensor_add(l2[:], l2[:], tmp[:])
            nc.vector.tensor_tensor(out=tmp[:], in0=d[2][:], in1=d[2][:], op=MUL)
            nc.vector.tensor_add(l2[:], l2[:], tmp[:])
            ll = s1.tile([128, 24], F32, tag="ll")
            nc.scalar.activation(ll[:], l2[:], SQRT)
            rinv2 = s1.tile([128, 24], F32, tag="rinv2")
            nc.vector.reciprocal(rinv2[:], l2[:])
            # smooth weight: uu=(l-rmin)/(rmax-rmin) clamped to [0,1],
            # vv = uu^3(-6uu^2+15uu-10)+1
            uu = s1.tile([128, 24], F32, tag="uu")
            sc = 1.0 / (RCUT - RCUT_SMTH)
            nc.scalar.activation(uu[:], ll[:], COPYF, bias=-RCUT_SMTH * sc, scale=sc)
            nc.vector.tensor_scalar(out=uu[:], in0=uu[:], scalar1=0.0, scalar2=1.0,
                                    op0=mybir.AluOpType.max, op1=mybir.AluOpType.min)
            poly = s1.tile([128, 24], F32, tag="poly")
            nc.vector.tensor_scalar(out=poly[:], in0=uu[:], scalar1=-6.0,
                                    scalar2=15.0, op0=MUL, op1=ADD)
            nc.vector.tensor_tensor(out=poly[:], in0=poly[:], in1=uu[:], op=MUL)
            nc.vector.tensor_scalar(out=poly[:], in0=poly[:], scalar1=10.0,
                                    scalar2=None, op0=SUB)
            u2 = s1.tile([128, 24], F32, tag="u2")
            nc.vector.tensor_tensor(out=u2[:], in0=uu[:], in1=uu[:], op=MUL)
            nc.vector.tensor_tensor(out=u2[:], in0=u2[:], in1=uu[:], op=MUL)
            nc.vector.tensor_tensor(out=poly[:], in0=poly[:], in1=u2[:], op=MUL)
            nc.vector.tensor_scalar(out=poly[:], in0=poly[:], scalar1=1.0,
                                    scalar2=None, op0=ADD)
            # s2 = sw / l^2 ; rr_c = (d_c * s2) * A_c - B_c
            s2 = s1.tile([128, 24], F32, tag="s2")
            nc.vector.tensor_tensor(out=s2[:], in0=poly[:], in1=rinv2[:], op=MUL)
            rrf = s1.tile([3, 3072], F32, tag="rrf")
            rrall = s1.tile([128, 72], F32, tag="rrall")
            for c in range(3):
                rr = rrall[:, 24 * c:24 * c + 24]
                nc.vector.tensor_tensor(out=rr, in0=d[c][:], in1=s2[:], op=MUL)
                nc.vector.tensor_tensor(out=rr, in0=rr,
                                        in1=Aexp[:, 24 * c:24 * c + 24], op=MUL)
                nc.vector.tensor_tensor(out=rr, in0=rr,
                                        in1=Bexp[:, 24 * c:24 * c + 24], op=SUB)
            for c in range(3):
                nc.sync.dma_start(rrf[c:c + 1, :], rrall[:, 24 * c:24 * c + 24])

            # pair matmuls -> envA (128 x 768) [4 atoms x (32 x 96) = env00|env01],
            #                 envB (128 x 1024) [2 atoms x (64 x 64) = env11]
            envA = s1.tile([128, 768], DT, tag="envA")
            envB = s1.tile([128, 1024], DT, tag="envB")
            for blk in range(8):          # 4-atom blocks
                pa = ps1.tile([128, 96], F32, tag="pa")
                for j in range(4):
                    a = 4 * blk + j
                    nc.tensor.matmul(
                        out=pa[32 * j:32 * j + 32, :],
                        lhsT=rrf[:, 96 * a:96 * a + 32],
                        rhs=rrf[:, 96 * a:96 * a + 96], start=True, stop=True,
                        tile_position=(0, 32 * j))
                nc.vector.tensor_copy(envA[:, 96 * blk:96 * blk + 96], pa[:])
            for blk in range(16):         # 2-atom blocks
                pb = ps1.tile([128, 64], F32, tag="pb")
                for j in range(2):
                    a = 2 * blk + j
                    nc.tensor.matmul(
                        out=pb[64 * j:64 * j + 64, :],
                        lhsT=rrf[:, 96 * a + 32:96 * a + 96],
                        rhs=rrf[:, 96 * a + 32:96 * a + 96], start=True,
                        stop=True, tile_position=(0, 64 * j))
                nc.vector.tensor_copy(envB[:, 64 * blk:64 * blk + 64], pb[:])

            # ---- flatten into the MLP batch order (plain row-major within
            # each (atom, pair) segment).
            # One DMA per (q, l, segment) covering both atoms aa=0,1: for
            # a = 8l+2q+aa the envA partition blocks of aa=0,1 are adjacent
            # (32*(a%4) pairs) and the col block 96*(a//4) is shared, so a
            # leading (aa i) partition split pairs with the 7168-strided aa
            # blocks of the env_mlp row. Split across the SP and ACT queues.
            env_mlp = s1.tile([128, 14336], DT, tag="env_mlp")
            for q in range(4):
                for l in range(4):
                    row = 32 * q + l
                    eng = nc.sync if l % 2 == 0 else nc.scalar
                    for aa in range(2):
                        a = 8 * l + 2 * q + aa
                        base = 7168 * aa
                        srcs = (
                            (0, envA[32 * (a % 4):32 * (a % 4) + 32,
                                     96 * (a // 4):96 * (a // 4) + 32]),
                            (1024, envA[32 * (a % 4):32 * (a % 4) + 32,
                                        96 * (a // 4) + 32:
                                        96 * (a // 4) + 96]),
                            (3072, envB[64 * (a % 2):64 * (a % 2) + 64,
                                        64 * (a // 2):64 * (a // 2) + 64]),
                        )
                        for off, src in srcs:
                            seglen = src.shape[0] * src.shape[1]
                            eng.dma_start(
                                env_mlp[row:row + 1, base + off:
                                        base + off + seglen], src)

            # ---- env_bm per lane (128 x 3584), contiguous window sections:
            # window w occupies cols [WBASE[w], WBASE[w]+8*NCHW[w]); section r
            # holds the chunks whose acc row is r, at their window position.
            # Built on PE: chunk-column block = tile^T @ S_k1 (selection),
            # with the normal env tile as lhsT (psum rows C*k1 via col tiling).
            env_bm = []
            for l in range(4):
                eb = s1.tile([128, 3584], DT, tag=f"env_bm{l}",
                             name=f"env_bm{l}")
                nc.vector.memset(eb[:], 0.0)
                env_bm.append(eb)
            for l in range(4):
                for ap_ in range(8):        # atom within lane
                    a = 8 * l + ap_
                    for P in range(3):
                        sid = 3 * ap_ + P
                        w = sid // 2
                        nch = SEG_NCH[P]
                        r = 4 * (sid % 2) + l
                        col0 = (WBASE[w] + r * NCHW[w]
                                + SID_START[sid] - W0[w])
                        if P == 0:
                            tile_ = envA[32 * (a % 4):32 * (a % 4) + 32,
                                         96 * (a // 4):96 * (a // 4) + 32]
                            k1c, C, rb = 4, 32, 32 * (a % 4)
                            sreps = s0rep
                        elif P == 1:
                            tile_ = envA[32 * (a % 4):32 * (a % 4) + 32,
                                         96 * (a // 4) + 32:96 * (a // 4) + 96]
                            k1c, C, rb = 2, 64, 32 * (a % 4)
                            sreps = s1rep
                        else:
                            tile_ = envB[64 * (a % 2):64 * (a % 2) + 64,
                                         64 * (a // 2):64 * (a // 2) + 64]
                            k1c, C, rb = 2, 64, 64 * (a % 2)
                            sreps = s2rep
                        R = tile_.shape[0]
                        pbm = psbm.tile([128, nch], F32, tag="pbm", name="pbm")
                        for kk in range(k1c):
                            nc.tensor.matmul(
                                out=pbm[C * kk:C * kk + C, :],
                                lhsT=tile_,
                                rhs=sreps[kk][rb:rb + R, :],
                                start=True, stop=True,
                                tile_position=(rb, C * kk))
                        nc.vector.tensor_copy(env_bm[l][:, col0:col0 + nch],
                                              pbm[:])

        # ---- stage 2: MLP + contraction over 112 supertiles
        with tc.tile_pool(name="psL1", bufs=1, space="PSUM") as psL1, \
             tc.tile_pool(name="psL2", bufs=1, space="PSUM") as psL2, \
             tc.tile_pool(name="psP3", bufs=2, space="PSUM") as psP3, \
             tc.tile_pool(name="psacc", bufs=1, space="PSUM") as psacc:

            acc_tile = None
            win_open = -1
            for s in range(NSUPER):
                q, si = s // 28, s % 28
                P = PAIR_OF_POS[s % 14]
                p1 = psL1.tile([128, 512], F32, tag="p1")
                nc.tensor.matmul(out=p1[:],
                                 lhsT=w1sb[P][32 * q:32 * q + 4, :],
                                 rhs=env_mlp[32 * q:32 * q + 4,
                                             512 * si:512 * si + 512],
                                 start=True, stop=True,
                                 tile_position=(32 * q, 0))
                t1 = sbT1.tile([128, 512], DT, tag="t1")
                nc.scalar.activation(t1[:], p1[:], TANH, bias=b1sb[P][:, 0:1])
                # T1dup (128 x 1024): cols 512t, rows: {0:32,32:64}<-T1[64t:+32],
                # {64:104}<-T1[64t+24:+40], {104:128}<-T1[64t+32:+24]
                # t1 group dup for the resnet residual: only rows {0:24,32:56}
                # (even elem) and {72:96,104:128} (odd elem) are read by the
                # adds (other rows of t1 are exact zeros); write each 24-row
                # source to its two 32-aligned destinations in one DMA via a
                # stride-0 source dim, spread across engine DGE queues.
                t1d = sbT1d.tile([128, 1024], DT, tag="t1d")
                for t in range(2):
                    for (dr0, nr, sr0) in ((0, 32, 0), (32, 32, 0),
                                           (64, 40, 24), (104, 24, 32)):
                        nc.gpsimd.dma_start(
                            t1d[dr0:dr0 + nr, 512 * t:512 * t + 512],
                            t1[64 * t + sr0:64 * t + sr0 + nr, :])
                p2 = psL2.tile([128, 1024], F32, tag="p2")
                for t in range(2):
                    nc.tensor.matmul(out=p2[:, 512 * t:512 * t + 512],
                                     lhsT=w2Fsb[P][64 * t:64 * t + 64, :],
                                     rhs=t1[64 * t:64 * t + 64, :],
                                     start=True, stop=True,
                                     tile_position=(64 * t, 0))
                t2all = sbT2.tile([128, 1024], DT, tag="t2")
                nc.scalar.activation(t2all[:], p2[:], TANH, bias=b2sb[P][:, 0:1])
                nc.vector.tensor_add(t2all[:], t2all[:], t1d[:])
                for l in range(4):
                    t, par = l // 2, l % 2
                    p3 = psP3.tile([128, 1024], F32, tag="p3")
                    for qq in range(4):
                        c0 = 512 * t + 128 * qq
                        if par == 0:
                            nc.tensor.matmul(
                                out=p3[:, 256 * qq:256 * qq + 144],
                                lhsT=t2all[0:65, c0:c0 + 128],
                                rhs=w3Asb[P][:], start=True, stop=True)
                        else:
                            nc.tensor.matmul(
                                out=p3[:, 256 * qq:256 * qq + 144],
                                lhsT=t2all[64:128, c0:c0 + 128],
                                rhs=w3Bsb[P][64:128, :], start=True, stop=True)
                    tt = sbTT.tile([128, 576], DT, tag="tt")
                    p3r = p3[:].rearrange("p (k x) -> p k x", k=4)
                    ttr = tt[:].rearrange("p (k x) -> p k x", k=4)
                    nc.scalar.activation(ttr[:, :, 0:96], p3r[:, :, 0:96], TANH)
                    nc.vector.tensor_copy(ttr[:, :, 96:144], p3r[:, :, 96:144])
                    for qq in range(4):
                        cc = 4 * s + qq
                        sid = _sid_of_cc(cc)
                        w = sid // 2
                        if w != win_open:
                            # flush previous window via SBUF staging
                            if acc_tile is not None:
                                stg = sbTT.tile([8, 144], F32, tag="stg",
                                                name="stg")
                                nc.vector.tensor_copy(stg[:], acc_tile[:])
                                nc.sync.dma_start(
                                    st_all[8 * win_open:8 * win_open + 8, :],
                                    stg[:])
                            acc_tile = psacc.tile([8, 144], F32, tag="acc")
                            win_open = w
                            first = True
                        else:
                            first = False
                        last = (sid % 2 == 1) and (cc == 56 * (sid // 3)
                                                   + (POFF[sid % 3] + PLEN[sid % 3]) // 128 - 1) \
                            and (l == 3)
                        # lhsT: 8 one-hot cols = window sections at this
                        # chunk's position (stride NCHW[w])
                        pos = cc - W0[w]
                        lhs = env_bm[l][:, WBASE[w]:WBASE[w] + 8 * NCHW[w]] \
                            .rearrange("p (r j) -> p j r", r=8)[:, pos, :]
                        nc.tensor.matmul(
                            out=acc_tile[:], lhsT=lhs,
                            rhs=tt[:, 144 * qq:144 * qq + 144],
                            start=first, stop=last, skip_group_check=True)
            # flush last window
            stg = sbTT.tile([8, 144], F32, tag="stg", name="stg")
            nc.vector.tensor_copy(stg[:], acc_tile[:])
            nc.sync.dma_start(st_all[8 * win_open:8 * win_open + 8, :], stg[:])

            # ---- stage 3: combine
            res_ps = psL1.tile([32, 144], F32, tag="p1")
            nc.tensor.matmul(out=res_ps[:], lhsT=selsb[:], rhs=st_all[:],
                             start=True, stop=True)
            res_cp = wpool.tile([32, 144], F32, tag="rescp")
            nc.vector.tensor_copy(res_cp[:], res_ps[:])
            res_sb = wpool.tile([32, 96], F32, tag="res")
            nc.vector.tensor_tensor(
                out=res_sb[:].rearrange("p (r f) -> p r f", r=2),
                in0=res_cp[:, 0:96].rearrange("p (r f) -> p r f", r=2),
                in1=res_cp[:, 96:144].rearrange("p (r f) -> p r f", r=1)
                    .to_broadcast([32, 2, 48]),
                op=ADD)
            nc.sync.dma_start(outD, res_sb[:])

    nc.compile()
    return nc


_CACHE = {}


def _get_program():
    if "nc" not in _CACHE:
        _CACHE["nc"] = build_program()
    return _CACHE["nc"]


def make_in_maps(nlist, extended_coord, extended_atype, mean, stddev,
                 W1, b1, W2, b2, W3, b3):
    nlist = np.asarray(nlist)
    aux = _build_static_aux(np.asarray(W1, np.float32), np.asarray(b1, np.float32),
                            np.asarray(W2, np.float32), np.asarray(b2, np.float32),
                            np.asarray(W3, np.float32), np.asarray(b3, np.float32),
                            np.asarray(mean, np.float32),
                            np.asarray(stddev, np.float32))
    coordT = np.asarray(extended_coord, np.float32).reshape(NALL, 3)
    atypeF = np.asarray(extended_atype).astype(np.float32).reshape(NALL, 1)

    in_maps = []
    for c in range(NCORES):
        m = {
            "coordT": coordT,
            "atypeF": atypeF,
            "nlist24": nlist[0, 32 * c:32 * c + 32, :].astype(np.int32)
                       .reshape(128, 24),
            "ownid": (32 * c + np.arange(128) // 4).astype(np.int32)
                     .reshape(128, 1),
            "qvec": aux["qvec"],
            "mstabA": aux["mstabA"], "mstabB": aux["mstabB"],
            "sel": aux["sel"],
            "s0rep": aux["s0rep"], "s1rep": aux["s1rep"],
            "s2rep": aux["s2rep"],
            "w1rep": aux["w1rep"], "b1t": aux["b1t"],
            "w2F": aux["w2F"], "b2sp": aux["b2sp"],
            "w3A": aux["w3A"], "w3B": aux["w3B"],
        }
        in_maps.append(m)
    return in_maps


def kernel_run(trace=False, **inputs):
    in_maps = make_in_maps(**inputs)
    nc = _get_program()
    res = run_bass_kernel_spmd(nc, in_maps, core_ids=list(range(NCORES)),
                               trace=trace)
    out = np.concatenate([res.results[c]["out"] for c in range(NCORES)], axis=0)
    return out.reshape(1, NLOC, NG).astype(np.float32), res


def kernel(nlist, extended_coord, extended_atype, mean, stddev,
           W1, b1, W2, b2, W3, b3):
    out, _ = kernel_run(
        nlist=nlist, extended_coord=extended_coord,
        extended_atype=extended_atype, mean=mean, stddev=stddev,
        W1=W1, b1=b1, W2=W2, b2=b2, W3=W3, b3=b3)
    return out

